# revision 41
# baseline (speedup 1.0000x reference)
"""nn_Attention: out[b,h] = strict_tril(rope(Q[b,h]) @ rope(Q[b,h])^T) @ V[b].

Sharding: one (b,h) pair per NeuronCore (B*H = 8 pairs on 8 cores, fully
data-parallel, no collectives).

Host-side staging de-interleaves Q's even/odd columns AND transposes it
(both pure relayouts: scores contract over all of n, so any fixed
n-permutation is mathematically neutral, and the transpose just picks
which axis lands on SBUF partitions), plus casts to bf16 (the kernel
cast-loaded to bf16 anyway).  RoPE is then computed DIRECTLY in the
QR^T chunk layout the score matmuls need as both lhsT and rhs - the
PE-transpose phase of the previous design (62us of PE time) disappears.

Per core, in waves of tq=512 t-columns:

  phase 0 : load qT pair-chunk tiles (Qe^T rows 0..n/2, Qo^T rows n/2..)
            and transposed cos/sin tables for the wave's t-range; RoPE on
            DVE with dense step-1 bf16 ops (2x mode):
              QRe_k = Qe_k*c_k - Qo_k*s_k -> qrt chunk k
              QRo_k = Qo_k*c_k + Qe_k*s_k -> qrt chunk 8+k
  phase A : score strips T_j = QR_j @ QR^T[:, lo:wave_end] (upper-triangle
            blocks only; scores are symmetric so T_ji doubles as the
            transposed lhsT for phase B), 512-wide f32 PSUM tiles
            (LDWEIGHTS fully hidden at this width), strict-upper mask on
            the diagonal block, cast to bf16 strips.
  phase B : out_i = sum_{j<=i} matmul(lhsT=T_ji, rhs=V_j) accumulated in
            PSUM, copied out as f32 and stored per row block.
"""

import math
from functools import lru_cache

import numpy as np
import ml_dtypes

import concourse.bass as bass
import concourse.mybir as mybir
import concourse.tile as tile
from concourse import bacc
from concourse.bass_utils import run_bass_kernel_spmd
from concourse.masks import make_upper_triangular

THETA = 2.0 ** 16
P = 128
TMODE = "tr"  # kept for test.py --tmode compat; unused

BF16 = mybir.dt.bfloat16
F32 = mybir.dt.float32


def _v_sched(bounds, nt):
    """Per wave, the V row-block range [jv0, jv1) to load: wave w loads
    exactly what its own phase B consumes (emitted before phase B, so it
    overlaps that wave's RoPE + phase A)."""
    nwv = len(bounds) - 1
    sched, done = [], 0
    for w in range(nwv):
        need = bounds[w + 1] // P
        if w == nwv - 1:
            need = nt
        sched.append((done, need))
        done = max(done, need)
    return sched


def _wave_block(mat, bounds):
    """[n, t] -> flat wave-blocked staging: per wave w (cols [c0, c1)),
    two contiguous half-blocks, each laid out [P, half, wsz] C-order
    (partition-major, matching the SBUF tile)."""
    n, t = mat.shape
    half = n // (2 * P)
    out = np.empty(n * t, dtype=mat.dtype)
    pos = 0
    for w in range(len(bounds) - 1):
        c0, c1 = bounds[w], bounds[w + 1]
        wsz = c1 - c0
        for hlf in range(2):
            blk = mat[hlf * n // 2 : (hlf + 1) * n // 2, c0:c1]
            blk = blk.reshape(half, P, wsz).transpose(1, 0, 2)
            out[pos : pos + blk.size] = blk.reshape(-1)
            pos += blk.size
    return out


@lru_cache(maxsize=None)
def _rope_tables(t, n):
    """Transposed cos/sin tables matching reference._rope, bf16,
    wave-blocked (cos half stacked over sin half per wave).

    cosT[p, t] = cos(phase[t, 2p]), sinT[p, t] = sin(phase[t, 2p]);
    one entry per pair (reference quantizes freqs in pairs).
    """
    idx = ((np.arange(n) // 2) * 2).astype(np.float32)
    freqs = (1.0 / (THETA ** (idx / np.float32(n))) / np.float32(2.0 * math.pi)).astype(
        np.float32
    )
    pos = np.arange(t, dtype=np.float32)[:, None]
    phases = ((pos * freqs) % np.float32(1.0)) * np.float32(2.0 * math.pi)
    cos_h = np.cos(phases)[:, 0::2]
    sin_h = np.sin(phases)[:, 0::2]
    cs = np.vstack([cos_h.T, sin_h.T]).astype(ml_dtypes.bfloat16)
    return _wave_block(np.ascontiguousarray(cs), _wave_bounds(t))


def _wave_bounds(t):
    """Wave column boundaries: small warmup waves (so the first DMAs land
    and the DVE RoPE for wave w+1 finishes before the PE drains wave w's
    matmuls), then 512-wide steady-state waves."""
    bounds = [0]
    for wsz in (128, 128, 256):
        if bounds[-1] + wsz <= t:
            bounds.append(bounds[-1] + wsz)
    while bounds[-1] < t:
        bounds.append(min(t, bounds[-1] + 512))
    return bounds


@lru_cache(maxsize=None)
def _build(t, n, d):
    from contextlib import ExitStack

    nt = t // P        # row blocks
    nk = n // P        # contraction chunks
    half = nk // 2     # pair chunks
    bounds = _wave_bounds(t)
    vsched = _v_sched(bounds, nt)
    assert n % (2 * P) == 0 and t % P == 0

    nc = bacc.Bacc("TRN2", target_bir_lowering=False, debug=False, num_swdge_queues=4)
    # qt/cs are wave-blocked on the host (see _stage_q/_rope_tables): for
    # each wave the [P, half, wsz] tile destined for each DMA is contiguous
    # partition-major, so every partition reads one multi-KB run (vs the
    # 256B-1KB strided segments a [n, t] layout would give).
    qt_d = nc.dram_tensor("qt", [n * t], BF16, kind="ExternalInput").ap()
    cs_d = nc.dram_tensor("cs", [n * t], BF16, kind="ExternalInput").ap()
    v_d = nc.dram_tensor("v", [t * d], BF16, kind="ExternalInput").ap()
    out_d = nc.dram_tensor("out", [t, d], F32, kind="ExternalOutput").ap()

    with tile.TileContext(nc) as tc, ExitStack() as ctx:
        const = ctx.enter_context(tc.tile_pool(name="const", bufs=1))
        umask = const.tile([P, P], BF16, name="umask")

        vpool = ctx.enter_context(tc.tile_pool(name="vpool", bufs=1))
        vb = vpool.tile([P, nt * d], BF16, name="vb")
        vb3 = vb.rearrange("p (j dd) -> p j dd", j=nt)

        qrt_pool = ctx.enter_context(tc.tile_pool(name="qrt_pool", bufs=1))
        # QR^T: chunk k ([n in [kP,(k+1)P)] x [t]) lives at cols [k*t,(k+1)*t)
        qrt = qrt_pool.tile([P, nk * t], BF16, name="qrt")

        strips_pool = ctx.enter_context(tc.tile_pool(name="strips", bufs=1))
        # strip j = S_j,(j..nt) = QR_j @ QR^T[:, jP:] as [s(128) x t(width)]
        strips = [
            strips_pool.tile([P, (nt - j) * P], BF16, name=f"strip{j}")
            for j in range(nt)
        ]

        qpool = ctx.enter_context(tc.tile_pool(name="qpool", bufs=2))
        cpool = ctx.enter_context(tc.tile_pool(name="cpool", bufs=3))
        tpool = ctx.enter_context(tc.tile_pool(name="tpool", bufs=1))
        outp = ctx.enter_context(tc.tile_pool(name="outp", bufs=3))

        spsum = ctx.enter_context(tc.tile_pool(name="spsum", bufs=6, space="PSUM"))
        opsum = ctx.enter_context(tc.tile_pool(name="opsum", bufs=2, space="PSUM"))

        qrt3 = qrt.rearrange("p (k tt) -> p k tt", k=nk)
        for w in range(len(bounds) - 1):
            c0, c1 = bounds[w], bounds[w + 1]
            wsz = c1 - c0
            # ---- phase 0: load + RoPE the wave's t-columns ---------------
            # spread each wave's loads over all three DMA paths so no single
            # ring/queue serializes the ramp: Qe half on the sync HWDGE
            # ring, Qo half on gpsimd SWDGE, cos/sin halves on the ACT
            # HWDGE ring.
            q2 = qpool.tile([P, nk, wsz], BF16, tag="q", name=f"q_{w}")
            cs2 = cpool.tile([P, nk, wsz], BF16, tag="cs", name=f"cs_{w}")
            qe2, qo2 = q2[:, 0:half, :], q2[:, half:nk, :]
            ct2, st2 = cs2[:, 0:half, :], cs2[:, half:nk, :]
            hb = (n // 2) * wsz  # elements per half-block
            ofs = n * c0         # wave block start in the flat staging
            qe_src = qt_d[ofs : ofs + hb].rearrange("(p kp c) -> p kp c", p=P, kp=half)
            qo_src = qt_d[ofs + hb : ofs + 2 * hb].rearrange(
                "(p kp c) -> p kp c", p=P, kp=half
            )
            ct_src = cs_d[ofs : ofs + hb].rearrange("(p kp c) -> p kp c", p=P, kp=half)
            st_src = cs_d[ofs + hb : ofs + 2 * hb].rearrange(
                "(p kp c) -> p kp c", p=P, kp=half
            )
            nc.sync.dma_start(out=qe2, in_=qe_src)
            nc.gpsimd.dma_start(out=qo2, in_=qo_src)
            nc.scalar.dma_start(out=ct2, in_=ct_src)
            nc.sync.dma_start(out=st2, in_=st_src)
            if w == 0:
                make_upper_triangular(nc, umask, val=1.0, diag=False)
            # V arrives incrementally (wave-blocked staging): phase B of
            # wave w only reads row blocks jj < c1/P, so each wave loads
            # just the blocks the NEXT wave needs, behind qo on the
            # SWDGE queue.
            jv0, jv1 = vsched[w]
            if jv1 > jv0:
                nc.gpsimd.dma_start(
                    out=vb3[:, jv0:jv1, :],
                    in_=v_d[jv0 * P * d : jv1 * P * d].rearrange(
                        "(p j dd) -> p j dd", p=P, j=jv1 - jv0
                    ),
                )
            # RoPE, batched across all pair chunks (6 big DVE ops per wave):
            #   QRe = Qe*c - Qo*s -> qrt chunks [0, half)
            #   QRo = Qo*c + Qe*s -> qrt chunks [half, nk)
            t1 = tpool.tile([P, half, wsz], BF16, tag="t1", name=f"t1_{w}")
            t2 = tpool.tile([P, half, wsz], BF16, tag="t2", name=f"t2_{w}")
            nc.vector.tensor_mul(t1, qe2, ct2)
            nc.vector.tensor_mul(t2, qo2, st2)
            nc.vector.tensor_sub(qrt3[:, 0:half, c0:c1], t1, t2)
            t3 = tpool.tile([P, half, wsz], BF16, tag="t1", name=f"t3_{w}")
            t4 = tpool.tile([P, half, wsz], BF16, tag="t2", name=f"t4_{w}")
            nc.vector.tensor_mul(t3, qo2, ct2)
            nc.vector.tensor_mul(t4, qe2, st2)
            nc.vector.tensor_add(qrt3[:, half:nk, c0:c1], t3, t4)

            # ---- phase A: score strip tiles landing in wave w ------------
            for j in range(c1 // P):
                lo = max(j * P, c0)
                hi = c1
                width = hi - lo
                ps = spsum.tile([P, width], F32, tag="ps", name=f"ps_{w}_{j}")
                for k in range(nk):
                    nc.tensor.matmul(
                        ps,
                        lhsT=qrt[:, k * t + j * P : k * t + (j + 1) * P],
                        rhs=qrt[:, k * t + lo : k * t + hi],
                        start=(k == 0),
                        stop=(k == nk - 1),
                    )
                l0 = lo - j * P
                if l0 == 0:
                    # diagonal block: strict upper triangle in [s,t]
                    nc.vector.tensor_mul(strips[j][:, 0:P], ps[:, 0:P], umask)
                    if width > P:
                        nc.scalar.copy(strips[j][:, P:width], ps[:, P:width])
                else:
                    nc.scalar.copy(strips[j][:, l0 : l0 + width], ps[:, :width])

            # ---- phase B: outputs for row blocks of wave w ---------------
            for i in range(c0 // P, c1 // P):
                po = opsum.tile([P, d], F32, tag="po", name=f"po_{i}")
                for jj in range(i + 1):
                    nc.tensor.matmul(
                        po,
                        lhsT=strips[jj][:, (i - jj) * P : (i - jj + 1) * P],
                        rhs=vb[:, jj * d : (jj + 1) * d],
                        start=(jj == 0),
                        stop=(jj == i),
                    )
                ot = outp.tile([P, d], F32, tag="ot", name=f"ot_{i}")
                nc.scalar.copy(ot, po)
                nc.scalar.dma_start(out=out_d[i * P : (i + 1) * P, :], in_=ot)

    nc.compile()
    return nc


def _stage_q(Qc):
    """(t, n) f32 -> flat bf16 staging: de-interleaved (Qe^T rows over
    Qo^T rows), transposed, wave-blocked."""
    t, n = Qc.shape
    qp = np.ascontiguousarray(Qc.reshape(t, n // 2, 2).transpose(2, 1, 0))
    qt = qp.reshape(n, t).astype(ml_dtypes.bfloat16)
    return _wave_block(qt, _wave_bounds(t))


def _stage_v(Vc):
    """(t, d) f32 -> flat bf16 staging, wave-blocked per _v_sched."""
    t, d = Vc.shape
    bounds = _wave_bounds(t)
    out = np.empty(t * d, dtype=ml_dtypes.bfloat16)
    for j0, j1 in _v_sched(bounds, t // P):
        if j1 > j0:
            blk = (
                Vc[j0 * P : j1 * P, :]
                .astype(ml_dtypes.bfloat16)
                .reshape(j1 - j0, P, d)
                .transpose(1, 0, 2)
            )
            out[j0 * P * d : j1 * P * d] = blk.reshape(-1)
    return out


def _run(Q, V, trace=False, **trace_kwargs):
    Q = np.asarray(Q, dtype=np.float32)
    V = np.asarray(V, dtype=np.float32)
    b, h, t, n = Q.shape
    d = V.shape[-1]
    ncores = b * h
    nc = _build(t, n, d)
    cs = _rope_tables(t, n)
    in_maps = []
    for core in range(ncores):
        bi, hi = divmod(core, h)
        in_maps.append(
            {
                "qt": _stage_q(Q[bi, hi]),
                "v": _stage_v(V[bi, 0]),
                "cs": cs,
            }
        )
    res = run_bass_kernel_spmd(
        nc, in_maps, core_ids=list(range(ncores)), trace=trace, **trace_kwargs
    )
    out = np.empty((b, h, t, d), dtype=np.float32)
    for core in range(ncores):
        bi, hi = divmod(core, h)
        out[bi, hi] = res.results[core]["out"]
    return out, res


def kernel(**inputs):
    out, _ = _run(inputs["Q"], inputs["V"], trace=False)
    return out


# revision 49
# speedup vs baseline: 1.0345x; 1.0345x over previous
"""nn_Attention: out[b,h] = strict_tril(rope(Q[b,h]) @ rope(Q[b,h])^T) @ V[b].

Sharding: one (b,h) pair per NeuronCore (B*H = 8 pairs on 8 cores, fully
data-parallel, no collectives).

Host-side staging de-interleaves Q's even/odd columns AND transposes it
(both pure relayouts: scores contract over all of n, so any fixed
n-permutation is mathematically neutral, and the transpose just picks
which axis lands on SBUF partitions), plus casts to bf16 (the kernel
cast-loaded to bf16 anyway).  RoPE is then computed DIRECTLY in the
QR^T chunk layout the score matmuls need as both lhsT and rhs - the
PE-transpose phase of the previous design (62us of PE time) disappears.

Per core, in waves of tq=512 t-columns:

  phase 0 : load qT pair-chunk tiles (Qe^T rows 0..n/2, Qo^T rows n/2..)
            and transposed cos/sin tables for the wave's t-range; RoPE on
            DVE with dense step-1 bf16 ops (2x mode):
              QRe_k = Qe_k*c_k - Qo_k*s_k -> qrt chunk k
              QRo_k = Qo_k*c_k + Qe_k*s_k -> qrt chunk 8+k
  phase A : score strips T_j = QR_j @ QR^T[:, lo:wave_end] (upper-triangle
            blocks only; scores are symmetric so T_ji doubles as the
            transposed lhsT for phase B), 512-wide f32 PSUM tiles
            (LDWEIGHTS fully hidden at this width), strict-upper mask on
            the diagonal block, cast to bf16 strips.
  phase B : out_i = sum_{j<=i} matmul(lhsT=T_ji, rhs=V_j) accumulated in
            PSUM, copied out as f32 and stored per row block.
"""

import math
from functools import lru_cache

import numpy as np
import ml_dtypes

import concourse.bass as bass
import concourse.mybir as mybir
import concourse.tile as tile
from concourse import bacc
from concourse.bass_utils import run_bass_kernel_spmd
from concourse.masks import make_upper_triangular

THETA = 2.0 ** 16
P = 128
TMODE = "tr"  # kept for test.py --tmode compat; unused

BF16 = mybir.dt.bfloat16
F32 = mybir.dt.float32


def _v_sched(bounds, nt):
    """Per wave, the V row-block range [jv0, jv1) to load: wave w loads
    exactly what its own phase B consumes (emitted before phase B, so it
    overlaps that wave's RoPE + phase A)."""
    nwv = len(bounds) - 1
    sched, done = [], 0
    for w in range(nwv):
        need = bounds[w + 1] // P
        if w == nwv - 1:
            need = nt
        sched.append((done, need))
        done = max(done, need)
    return sched


def _pair_freqs(t, n):
    idx = ((np.arange(n) // 2) * 2).astype(np.float32)
    freqs = (1.0 / (THETA ** (idx / np.float32(n))) / np.float32(2.0 * math.pi)).astype(
        np.float32
    )
    return freqs[0::2]  # (n/2,) one per pair


def _gen_k0(t, n):
    """First pair-chunk whose cos/sin tables are generated on-device via
    the ACT Sin table instead of DMA'd: needs the in-wave phase span
    2*pi*f*wsz to stay within the Sin table's accurate range."""
    fp = _pair_freqs(t, n)
    half = n // (2 * P)
    wszmax = max(
        b - a for a, b in zip(_wave_bounds(t), _wave_bounds(t)[1:])
    )
    k0 = half
    for k in range(half - 1, -1, -1):
        adv = 2.0 * math.pi * float(fp[k * P]) * wszmax
        if adv <= 0.5:
            k0 = k
        else:
            break
    return k0


def _wave_block(mat, bounds, keep=None):
    """[n, t] -> flat wave-blocked staging: per wave w (cols [c0, c1)),
    two contiguous half-blocks, each laid out [P, rows, wsz] C-order
    (partition-major, matching the SBUF tile).  `keep` limits each half
    to its first keep*P rows (tables whose tail chunks are generated
    on-device)."""
    n, t = mat.shape
    half = n // (2 * P)
    kp = half if keep is None else keep
    out = np.empty(2 * kp * P * t, dtype=mat.dtype)
    pos = 0
    for w in range(len(bounds) - 1):
        c0, c1 = bounds[w], bounds[w + 1]
        wsz = c1 - c0
        for hlf in range(2):
            blk = mat[hlf * n // 2 : hlf * n // 2 + kp * P, c0:c1]
            blk = blk.reshape(kp, P, wsz).transpose(1, 0, 2)
            out[pos : pos + blk.size] = blk.reshape(-1)
            pos += blk.size
    return out


@lru_cache(maxsize=None)
def _rope_tables(t, n):
    """Transposed cos/sin tables matching reference._rope, bf16,
    wave-blocked (cos half stacked over sin half per wave), truncated to
    the DMA'd chunks [0, gen_k0).

    cosT[p, t] = cos(phase[t, 2p]), sinT[p, t] = sin(phase[t, 2p]);
    one entry per pair (reference quantizes freqs in pairs).
    """
    fp = _pair_freqs(t, n)
    pos = np.arange(t, dtype=np.float32)[:, None]
    phases = ((pos * fp[None, :]) % np.float32(1.0)) * np.float32(2.0 * math.pi)
    cs = np.vstack([np.cos(phases).T, np.sin(phases).T]).astype(ml_dtypes.bfloat16)
    return _wave_block(
        np.ascontiguousarray(cs), _wave_bounds(t), keep=_gen_k0(t, n)
    )


@lru_cache(maxsize=None)
def _gen_tables(t, n):
    """Host-side parameters for on-device table generation of chunks
    [gen_k0, half): per (wave, table, chunk) a range-reduced bias at the
    wave's center column plus a per-pair scale, so the ACT Sin argument
    scale*c + bias stays within the accurate range.

    Returns (iota[P*512] f32, scale[(half-k0)*P] f32,
             phi[nw*2*(half-k0)*P] f32)."""
    fp = _pair_freqs(t, n)
    bounds = _wave_bounds(t)
    half = n // (2 * P)
    k0 = _gen_k0(t, n)
    ng = half - k0
    iota = np.tile(np.arange(512, dtype=np.float32), (P, 1)).reshape(-1)
    sc = np.empty((P, ng), dtype=np.float32)
    for k in range(k0, half):
        sc[:, k - k0] = 2.0 * np.pi * fp[k * P : (k + 1) * P]
    phi = np.empty((P, (len(bounds) - 1) * 2 * ng), dtype=np.float32)
    twopi = 2.0 * np.pi
    for w in range(len(bounds) - 1):
        c0, c1 = bounds[w], bounds[w + 1]
        mid = (c1 - c0) // 2
        for tau in range(2):  # 0 = cos (sin(x + pi/2)), 1 = sin
            for k in range(k0, half):
                f = fp[k * P : (k + 1) * P].astype(np.float64)
                ph = twopi * ((c0 + mid) * f % 1.0) + (np.pi / 2 if tau == 0 else 0.0)
                ph = (ph + np.pi) % twopi - np.pi      # reduce to [-pi, pi)
                ph = ph - twopi * f * mid              # shift back to c=0
                col = (w * 2 + tau) * ng + (k - k0)
                phi[:, col] = ph.astype(np.float32)
    return iota, sc.reshape(-1), phi.reshape(-1)


def _wave_bounds(t):
    """Wave column boundaries: small warmup waves (so the first DMAs land
    and the DVE RoPE for wave w+1 finishes before the PE drains wave w's
    matmuls), then 512-wide steady-state waves."""
    bounds = [0]
    for wsz in (128, 128, 256):
        if bounds[-1] + wsz <= t:
            bounds.append(bounds[-1] + wsz)
    while bounds[-1] < t:
        bounds.append(min(t, bounds[-1] + 512))
    return bounds


@lru_cache(maxsize=None)
def _build(t, n, d):
    from contextlib import ExitStack

    nt = t // P        # row blocks
    nk = n // P        # contraction chunks
    half = nk // 2     # pair chunks
    bounds = _wave_bounds(t)
    vsched = _v_sched(bounds, nt)
    gk0 = _gen_k0(t, n)   # chunks [gk0, half) of each table half: ACT-generated
    ng = half - gk0
    assert n % (2 * P) == 0 and t % P == 0

    nc = bacc.Bacc("TRN2", target_bir_lowering=False, debug=False, num_swdge_queues=4)
    # qt/cs are wave-blocked on the host (see _stage_q/_rope_tables): for
    # each wave the [P, half, wsz] tile destined for each DMA is contiguous
    # partition-major, so every partition reads one multi-KB run (vs the
    # 256B-1KB strided segments a [n, t] layout would give).
    qt_d = nc.dram_tensor("qt", [n * t], BF16, kind="ExternalInput").ap()
    cs_d = nc.dram_tensor("cs", [2 * gk0 * P * t], BF16, kind="ExternalInput").ap()
    v_d = nc.dram_tensor("v", [t * d], BF16, kind="ExternalInput").ap()
    nw = len(bounds) - 1
    if ng > 0:
        iota_d = nc.dram_tensor("giota", [P * 512], F32, kind="ExternalInput").ap()
        gsc_d = nc.dram_tensor("gsc", [P * ng], F32, kind="ExternalInput").ap()
        gphi_d = nc.dram_tensor("gphi", [P * nw * 2 * ng], F32, kind="ExternalInput").ap()
    out_d = nc.dram_tensor("out", [t, d], F32, kind="ExternalOutput").ap()

    with tile.TileContext(nc) as tc, ExitStack() as ctx:
        const = ctx.enter_context(tc.tile_pool(name="const", bufs=1))
        umask = const.tile([P, P], BF16, name="umask")
        if ng > 0:
            iota2 = const.tile([P, 512], F32, name="giota")
            gsc = const.tile([P, ng], F32, name="gsc")
            gphi = const.tile([P, nw * 2 * ng], F32, name="gphi")

        vpool = ctx.enter_context(tc.tile_pool(name="vpool", bufs=1))
        vb = vpool.tile([P, nt * d], BF16, name="vb")
        vb3 = vb.rearrange("p (j dd) -> p j dd", j=nt)

        qrt_pool = ctx.enter_context(tc.tile_pool(name="qrt_pool", bufs=1))
        # QR^T: chunk k ([n in [kP,(k+1)P)] x [t]) lives at cols [k*t,(k+1)*t)
        qrt = qrt_pool.tile([P, nk * t], BF16, name="qrt")

        strips_pool = ctx.enter_context(tc.tile_pool(name="strips", bufs=1))
        # strip j = S_j,(j..nt) = QR_j @ QR^T[:, jP:] as [s(128) x t(width)]
        strips = [
            strips_pool.tile([P, (nt - j) * P], BF16, name=f"strip{j}")
            for j in range(nt)
        ]

        qpool = ctx.enter_context(tc.tile_pool(name="qpool", bufs=2))
        cpool = ctx.enter_context(tc.tile_pool(name="cpool", bufs=3))
        tpool = ctx.enter_context(tc.tile_pool(name="tpool", bufs=1))
        outp = ctx.enter_context(tc.tile_pool(name="outp", bufs=3))

        spsum = ctx.enter_context(tc.tile_pool(name="spsum", bufs=6, space="PSUM"))
        opsum = ctx.enter_context(tc.tile_pool(name="opsum", bufs=2, space="PSUM"))

        qrt3 = qrt.rearrange("p (k tt) -> p k tt", k=nk)
        for w in range(len(bounds) - 1):
            c0, c1 = bounds[w], bounds[w + 1]
            wsz = c1 - c0
            # ---- phase 0: load + RoPE the wave's t-columns ---------------
            # spread each wave's loads over all three DMA paths so no single
            # ring/queue serializes the ramp: Qe half on the sync HWDGE
            # ring, Qo half on gpsimd SWDGE, cos/sin halves on the ACT
            # HWDGE ring.
            q2 = qpool.tile([P, nk, wsz], BF16, tag="q", name=f"q_{w}")
            cs2 = cpool.tile([P, nk, wsz], BF16, tag="cs", name=f"cs_{w}")
            qe2, qo2 = q2[:, 0:half, :], q2[:, half:nk, :]
            ct2, st2 = cs2[:, 0:half, :], cs2[:, half:nk, :]
            hb = (n // 2) * wsz   # elements per q half-block
            gb = gk0 * P * wsz    # elements per DMA'd table half-block
            ofs = n * c0          # q wave block start in the flat staging
            cofs = 2 * gk0 * P * c0
            qe_src = qt_d[ofs : ofs + hb].rearrange("(p kp c) -> p kp c", p=P, kp=half)
            qo_src = qt_d[ofs + hb : ofs + 2 * hb].rearrange(
                "(p kp c) -> p kp c", p=P, kp=half
            )
            ct_src = cs_d[cofs : cofs + gb].rearrange(
                "(p kp c) -> p kp c", p=P, kp=gk0
            )
            st_src = cs_d[cofs + gb : cofs + 2 * gb].rearrange(
                "(p kp c) -> p kp c", p=P, kp=gk0
            )
            nc.sync.dma_start(out=qe2, in_=qe_src)
            nc.gpsimd.dma_start(out=qo2, in_=qo_src)
            nc.scalar.dma_start(out=ct2[:, 0:gk0, :], in_=ct_src)
            nc.sync.dma_start(out=st2[:, 0:gk0, :], in_=st_src)
            if w == 0 and ng > 0:
                nc.sync.dma_start(
                    out=iota2, in_=iota_d.rearrange("(p c) -> p c", p=P)
                )
                nc.scalar.dma_start(
                    out=gsc, in_=gsc_d.rearrange("(p c) -> p c", p=P)
                )
                nc.scalar.dma_start(
                    out=gphi, in_=gphi_d.rearrange("(p c) -> p c", p=P)
                )
            # generate the low-frequency table chunks on the ACT engine:
            # table[p, c] = Sin(scale_p * c + phi_p) with phi range-reduced
            # at the wave's center column on the host
            for kc in range(gk0, half):
                for tau in range(2):  # 0 = cos half, 1 = sin half
                    col = (w * 2 + tau) * ng + (kc - gk0)
                    nc.scalar.activation(
                        cs2[:, tau * half + kc, :],
                        iota2[:, 0:wsz],
                        mybir.ActivationFunctionType.Sin,
                        bias=gphi[:, col : col + 1],
                        scale=gsc[:, kc - gk0 : kc - gk0 + 1],
                    )
            if w == 0:
                make_upper_triangular(nc, umask, val=1.0, diag=False)
            # V arrives incrementally (wave-blocked staging): phase B of
            # wave w only reads row blocks jj < c1/P, so each wave loads
            # just the blocks the NEXT wave needs, behind qo on the
            # SWDGE queue.
            jv0, jv1 = vsched[w]
            if jv1 > jv0:
                nc.gpsimd.dma_start(
                    out=vb3[:, jv0:jv1, :],
                    in_=v_d[jv0 * P * d : jv1 * P * d].rearrange(
                        "(p j dd) -> p j dd", p=P, j=jv1 - jv0
                    ),
                )
            # RoPE, batched across all pair chunks (6 big DVE ops per wave):
            #   QRe = Qe*c - Qo*s -> qrt chunks [0, half)
            #   QRo = Qo*c + Qe*s -> qrt chunks [half, nk)
            t1 = tpool.tile([P, half, wsz], BF16, tag="t1", name=f"t1_{w}")
            t2 = tpool.tile([P, half, wsz], BF16, tag="t2", name=f"t2_{w}")
            nc.vector.tensor_mul(t1, qe2, ct2)
            nc.vector.tensor_mul(t2, qo2, st2)
            nc.vector.tensor_sub(qrt3[:, 0:half, c0:c1], t1, t2)
            t3 = tpool.tile([P, half, wsz], BF16, tag="t1", name=f"t3_{w}")
            t4 = tpool.tile([P, half, wsz], BF16, tag="t2", name=f"t4_{w}")
            nc.vector.tensor_mul(t3, qo2, ct2)
            nc.vector.tensor_mul(t4, qe2, st2)
            nc.vector.tensor_add(qrt3[:, half:nk, c0:c1], t3, t4)

            # ---- phase A: score strip tiles landing in wave w ------------
            for j in range(c1 // P):
                lo = max(j * P, c0)
                hi = c1
                width = hi - lo
                ps = spsum.tile([P, width], F32, tag="ps", name=f"ps_{w}_{j}")
                for k in range(nk):
                    nc.tensor.matmul(
                        ps,
                        lhsT=qrt[:, k * t + j * P : k * t + (j + 1) * P],
                        rhs=qrt[:, k * t + lo : k * t + hi],
                        start=(k == 0),
                        stop=(k == nk - 1),
                    )
                l0 = lo - j * P
                if l0 == 0:
                    # diagonal block: strict upper triangle in [s,t]
                    nc.vector.tensor_mul(strips[j][:, 0:P], ps[:, 0:P], umask)
                    if width > P:
                        nc.scalar.copy(strips[j][:, P:width], ps[:, P:width])
                else:
                    nc.scalar.copy(strips[j][:, l0 : l0 + width], ps[:, :width])

            # ---- phase B: outputs for row blocks of wave w ---------------
            for i in range(c0 // P, c1 // P):
                po = opsum.tile([P, d], F32, tag="po", name=f"po_{i}")
                for jj in range(i + 1):
                    nc.tensor.matmul(
                        po,
                        lhsT=strips[jj][:, (i - jj) * P : (i - jj + 1) * P],
                        rhs=vb[:, jj * d : (jj + 1) * d],
                        start=(jj == 0),
                        stop=(jj == i),
                    )
                ot = outp.tile([P, d], F32, tag="ot", name=f"ot_{i}")
                nc.scalar.copy(ot, po)
                nc.scalar.dma_start(out=out_d[i * P : (i + 1) * P, :], in_=ot)

    nc.compile()
    return nc


def _stage_q(Qc):
    """(t, n) f32 -> flat bf16 staging: de-interleaved (Qe^T rows over
    Qo^T rows), transposed, wave-blocked."""
    t, n = Qc.shape
    qp = np.ascontiguousarray(Qc.reshape(t, n // 2, 2).transpose(2, 1, 0))
    qt = qp.reshape(n, t).astype(ml_dtypes.bfloat16)
    return _wave_block(qt, _wave_bounds(t))


def _stage_v(Vc):
    """(t, d) f32 -> flat bf16 staging, wave-blocked per _v_sched."""
    t, d = Vc.shape
    bounds = _wave_bounds(t)
    out = np.empty(t * d, dtype=ml_dtypes.bfloat16)
    for j0, j1 in _v_sched(bounds, t // P):
        if j1 > j0:
            blk = (
                Vc[j0 * P : j1 * P, :]
                .astype(ml_dtypes.bfloat16)
                .reshape(j1 - j0, P, d)
                .transpose(1, 0, 2)
            )
            out[j0 * P * d : j1 * P * d] = blk.reshape(-1)
    return out


def _run(Q, V, trace=False, **trace_kwargs):
    Q = np.asarray(Q, dtype=np.float32)
    V = np.asarray(V, dtype=np.float32)
    b, h, t, n = Q.shape
    d = V.shape[-1]
    ncores = b * h
    nc = _build(t, n, d)
    cs = _rope_tables(t, n)
    gen = {}
    if _gen_k0(t, n) < n // (2 * P):
        iota, gsc, gphi = _gen_tables(t, n)
        gen = {"giota": iota, "gsc": gsc, "gphi": gphi}
    in_maps = []
    for core in range(ncores):
        bi, hi = divmod(core, h)
        in_maps.append(
            {
                "qt": _stage_q(Q[bi, hi]),
                "v": _stage_v(V[bi, 0]),
                "cs": cs,
                **gen,
            }
        )
    res = run_bass_kernel_spmd(
        nc, in_maps, core_ids=list(range(ncores)), trace=trace, **trace_kwargs
    )
    out = np.empty((b, h, t, d), dtype=np.float32)
    for core in range(ncores):
        bi, hi = divmod(core, h)
        out[bi, hi] = res.results[core]["out"]
    return out, res


def kernel(**inputs):
    out, _ = _run(inputs["Q"], inputs["V"], trace=False)
    return out


# revision 51
# speedup vs baseline: 1.0535x; 1.0184x over previous
"""nn_Attention: out[b,h] = strict_tril(rope(Q[b,h]) @ rope(Q[b,h])^T) @ V[b].

Sharding: one (b,h) pair per NeuronCore (B*H = 8 pairs on 8 cores, fully
data-parallel, no collectives).

Host-side staging de-interleaves Q's even/odd columns AND transposes it
(both pure relayouts: scores contract over all of n, so any fixed
n-permutation is mathematically neutral, and the transpose just picks
which axis lands on SBUF partitions), plus casts to bf16 (the kernel
cast-loaded to bf16 anyway).  RoPE is then computed DIRECTLY in the
QR^T chunk layout the score matmuls need as both lhsT and rhs - the
PE-transpose phase of the previous design (62us of PE time) disappears.

Per core, in waves of tq=512 t-columns:

  phase 0 : load qT pair-chunk tiles (Qe^T rows 0..n/2, Qo^T rows n/2..)
            and transposed cos/sin tables for the wave's t-range; RoPE on
            DVE with dense step-1 bf16 ops (2x mode):
              QRe_k = Qe_k*c_k - Qo_k*s_k -> qrt chunk k
              QRo_k = Qo_k*c_k + Qe_k*s_k -> qrt chunk 8+k
  phase A : score strips T_j = QR_j @ QR^T[:, lo:wave_end] (upper-triangle
            blocks only; scores are symmetric so T_ji doubles as the
            transposed lhsT for phase B), 512-wide f32 PSUM tiles
            (LDWEIGHTS fully hidden at this width), strict-upper mask on
            the diagonal block, cast to bf16 strips.
  phase B : out_i = sum_{j<=i} matmul(lhsT=T_ji, rhs=V_j) accumulated in
            PSUM, copied out as f32 and stored per row block.
"""

import math
from functools import lru_cache

import numpy as np
import ml_dtypes

import concourse.bass as bass
import concourse.mybir as mybir
import concourse.tile as tile
from concourse import bacc
from concourse.bass_utils import run_bass_kernel_spmd
from concourse.masks import make_upper_triangular

THETA = 2.0 ** 16
P = 128
TMODE = "tr"  # kept for test.py --tmode compat; unused

BF16 = mybir.dt.bfloat16
F32 = mybir.dt.float32


def _v_sched(bounds, nt):
    """Per wave, the V row-block range [jv0, jv1) to load: wave w loads
    exactly what its own phase B consumes (emitted before phase B, so it
    overlaps that wave's RoPE + phase A)."""
    nwv = len(bounds) - 1
    sched, done = [], 0
    for w in range(nwv):
        need = bounds[w + 1] // P
        if w == nwv - 1:
            need = nt
        sched.append((done, need))
        done = max(done, need)
    return sched


def _pair_freqs(t, n):
    idx = ((np.arange(n) // 2) * 2).astype(np.float32)
    freqs = (1.0 / (THETA ** (idx / np.float32(n))) / np.float32(2.0 * math.pi)).astype(
        np.float32
    )
    return freqs[0::2]  # (n/2,) one per pair


def _gen_k0(t, n):
    """First pair-chunk whose cos/sin tables are generated on-device via
    the ACT Sin table instead of DMA'd: needs the in-wave phase span
    2*pi*f*wsz to stay within the Sin table's accurate range."""
    fp = _pair_freqs(t, n)
    half = n // (2 * P)
    wszmax = max(
        b - a for a, b in zip(_wave_bounds(t), _wave_bounds(t)[1:])
    )
    k0 = half
    for k in range(half - 1, -1, -1):
        adv = 2.0 * math.pi * float(fp[k * P]) * wszmax
        if adv <= 0.5:
            k0 = k
        else:
            break
    return k0


def _wave_block(mat, bounds, keep=None):
    """[n, t] -> flat wave-blocked staging: per wave w (cols [c0, c1)),
    two contiguous half-blocks, each laid out [P, rows, wsz] C-order
    (partition-major, matching the SBUF tile).  `keep` limits each half
    to its first keep*P rows (tables whose tail chunks are generated
    on-device)."""
    n, t = mat.shape
    half = n // (2 * P)
    kp = half if keep is None else keep
    out = np.empty(2 * kp * P * t, dtype=mat.dtype)
    pos = 0
    for w in range(len(bounds) - 1):
        c0, c1 = bounds[w], bounds[w + 1]
        wsz = c1 - c0
        for hlf in range(2):
            blk = mat[hlf * n // 2 : hlf * n // 2 + kp * P, c0:c1]
            blk = blk.reshape(kp, P, wsz).transpose(1, 0, 2)
            out[pos : pos + blk.size] = blk.reshape(-1)
            pos += blk.size
    return out


@lru_cache(maxsize=None)
def _rope_tables(t, n):
    """Transposed cos/sin tables matching reference._rope, bf16,
    wave-blocked (cos half stacked over sin half per wave), truncated to
    the DMA'd chunks [0, gen_k0).

    cosT[p, t] = cos(phase[t, 2p]), sinT[p, t] = sin(phase[t, 2p]);
    one entry per pair (reference quantizes freqs in pairs).
    """
    fp = _pair_freqs(t, n)
    pos = np.arange(t, dtype=np.float32)[:, None]
    phases = ((pos * fp[None, :]) % np.float32(1.0)) * np.float32(2.0 * math.pi)
    cs = np.vstack([np.cos(phases).T, np.sin(phases).T]).astype(ml_dtypes.bfloat16)
    return _wave_block(
        np.ascontiguousarray(cs), _wave_bounds(t), keep=_gen_k0(t, n)
    )


@lru_cache(maxsize=None)
def _gen_tables(t, n):
    """Host-side parameters for on-device table generation of chunks
    [gen_k0, half): per (wave, table, chunk) a range-reduced bias at the
    wave's center column plus a per-pair scale, so the ACT Sin argument
    scale*c + bias stays within the accurate range.

    Returns (iota[P*512] f32, scale[(half-k0)*P] f32,
             phi[nw*2*(half-k0)*P] f32)."""
    fp = _pair_freqs(t, n)
    bounds = _wave_bounds(t)
    half = n // (2 * P)
    k0 = _gen_k0(t, n)
    ng = half - k0
    iota = np.tile(np.arange(512, dtype=np.float32), (P, 1)).reshape(-1)
    sc = np.empty((P, ng), dtype=np.float32)
    for k in range(k0, half):
        sc[:, k - k0] = 2.0 * np.pi * fp[k * P : (k + 1) * P]
    phi = np.empty((P, (len(bounds) - 1) * 2 * ng), dtype=np.float32)
    twopi = 2.0 * np.pi
    for w in range(len(bounds) - 1):
        c0, c1 = bounds[w], bounds[w + 1]
        mid = (c1 - c0) // 2
        for tau in range(2):  # 0 = cos (sin(x + pi/2)), 1 = sin
            for k in range(k0, half):
                f = fp[k * P : (k + 1) * P].astype(np.float64)
                ph = twopi * ((c0 + mid) * f % 1.0) + (np.pi / 2 if tau == 0 else 0.0)
                ph = (ph + np.pi) % twopi - np.pi      # reduce to [-pi, pi)
                ph = ph - twopi * f * mid              # shift back to c=0
                col = (w * 2 + tau) * ng + (k - k0)
                phi[:, col] = ph.astype(np.float32)
    return iota, sc.reshape(-1), phi.reshape(-1)


def _wave_bounds(t):
    """Wave column boundaries: small warmup waves (so the first DMAs land
    and the DVE RoPE for wave w+1 finishes before the PE drains wave w's
    matmuls), then 512-wide steady-state waves."""
    bounds = [0]
    for wsz in (128, 128, 256):
        if bounds[-1] + wsz <= t:
            bounds.append(bounds[-1] + wsz)
    while bounds[-1] < t:
        bounds.append(min(t, bounds[-1] + 512))
    return bounds


@lru_cache(maxsize=None)
def _build(t, n, d):
    from contextlib import ExitStack

    nt = t // P        # row blocks
    nk = n // P        # contraction chunks
    half = nk // 2     # pair chunks
    bounds = _wave_bounds(t)
    vsched = _v_sched(bounds, nt)
    gk0 = _gen_k0(t, n)   # chunks [gk0, half) of each table half: ACT-generated
    ng = half - gk0
    assert n % (2 * P) == 0 and t % P == 0

    nc = bacc.Bacc("TRN2", target_bir_lowering=False, debug=False, num_swdge_queues=4)
    # qt/cs are wave-blocked on the host (see _stage_q/_rope_tables): for
    # each wave the [P, half, wsz] tile destined for each DMA is contiguous
    # partition-major, so every partition reads one multi-KB run (vs the
    # 256B-1KB strided segments a [n, t] layout would give).
    qt_d = nc.dram_tensor("qt", [n * t], BF16, kind="ExternalInput").ap()
    cs_d = nc.dram_tensor("cs", [2 * gk0 * P * t], BF16, kind="ExternalInput").ap()
    v_d = nc.dram_tensor("v", [t * d], BF16, kind="ExternalInput").ap()
    nw = len(bounds) - 1
    if ng > 0:
        iota_d = nc.dram_tensor("giota", [P * 512], F32, kind="ExternalInput").ap()
        gsc_d = nc.dram_tensor("gsc", [P * ng], F32, kind="ExternalInput").ap()
        gphi_d = nc.dram_tensor("gphi", [P * nw * 2 * ng], F32, kind="ExternalInput").ap()
    out_d = nc.dram_tensor("out", [t, d], F32, kind="ExternalOutput").ap()

    with tile.TileContext(nc) as tc, ExitStack() as ctx:
        const = ctx.enter_context(tc.tile_pool(name="const", bufs=1))
        umask = const.tile([P, P], BF16, name="umask")
        if ng > 0:
            iota2 = const.tile([P, 512], F32, name="giota")
            gsc = const.tile([P, ng], F32, name="gsc")
            gphi = const.tile([P, nw * 2 * ng], F32, name="gphi")

        vpool = ctx.enter_context(tc.tile_pool(name="vpool", bufs=1))
        vb = vpool.tile([P, nt * d], BF16, name="vb")
        vb3 = vb.rearrange("p (j dd) -> p j dd", j=nt)

        qrt_pool = ctx.enter_context(tc.tile_pool(name="qrt_pool", bufs=1))
        # QR^T: chunk k ([n in [kP,(k+1)P)] x [t]) lives at cols [k*t,(k+1)*t)
        qrt = qrt_pool.tile([P, nk * t], BF16, name="qrt")

        strips_pool = ctx.enter_context(tc.tile_pool(name="strips", bufs=1))
        # strip j = S_j,(j..nt) = QR_j @ QR^T[:, jP:] as [s(128) x t(width)]
        strips = [
            strips_pool.tile([P, (nt - j) * P], BF16, name=f"strip{j}")
            for j in range(nt)
        ]

        qpool = ctx.enter_context(tc.tile_pool(name="qpool", bufs=2))
        cpool = ctx.enter_context(tc.tile_pool(name="cpool", bufs=3))
        tpool = ctx.enter_context(tc.tile_pool(name="tpool", bufs=1))
        outp = ctx.enter_context(tc.tile_pool(name="outp", bufs=3))

        spsum = ctx.enter_context(tc.tile_pool(name="spsum", bufs=6, space="PSUM"))
        opsum = ctx.enter_context(tc.tile_pool(name="opsum", bufs=2, space="PSUM"))

        qrt3 = qrt.rearrange("p (k tt) -> p k tt", k=nk)
        if ng > 0:
            # table-gen parameters first: wave 0's RoPE depends on the
            # generated chunks, which depend on these tiny loads
            nc.sync.dma_start(out=iota2, in_=iota_d.rearrange("(p c) -> p c", p=P))
            nc.scalar.dma_start(out=gsc, in_=gsc_d.rearrange("(p c) -> p c", p=P))
            nc.scalar.dma_start(out=gphi, in_=gphi_d.rearrange("(p c) -> p c", p=P))
        for w in range(len(bounds) - 1):
            c0, c1 = bounds[w], bounds[w + 1]
            wsz = c1 - c0
            # ---- phase 0: load + RoPE the wave's t-columns ---------------
            # spread each wave's loads over all three DMA paths so no single
            # ring/queue serializes the ramp: Qe half on the sync HWDGE
            # ring, Qo half on gpsimd SWDGE, cos/sin halves on the ACT
            # HWDGE ring.
            q2 = qpool.tile([P, nk, wsz], BF16, tag="q", name=f"q_{w}")
            cs2 = cpool.tile([P, nk, wsz], BF16, tag="cs", name=f"cs_{w}")
            qe2, qo2 = q2[:, 0:half, :], q2[:, half:nk, :]
            ct2, st2 = cs2[:, 0:half, :], cs2[:, half:nk, :]
            hb = (n // 2) * wsz   # elements per q half-block
            gb = gk0 * P * wsz    # elements per DMA'd table half-block
            ofs = n * c0          # q wave block start in the flat staging
            cofs = 2 * gk0 * P * c0
            qe_src = qt_d[ofs : ofs + hb].rearrange("(p kp c) -> p kp c", p=P, kp=half)
            qo_src = qt_d[ofs + hb : ofs + 2 * hb].rearrange(
                "(p kp c) -> p kp c", p=P, kp=half
            )
            ct_src = cs_d[cofs : cofs + gb].rearrange(
                "(p kp c) -> p kp c", p=P, kp=gk0
            )
            st_src = cs_d[cofs + gb : cofs + 2 * gb].rearrange(
                "(p kp c) -> p kp c", p=P, kp=gk0
            )
            nc.sync.dma_start(out=qe2, in_=qe_src)
            nc.gpsimd.dma_start(out=qo2, in_=qo_src)
            nc.scalar.dma_start(out=ct2[:, 0:gk0, :], in_=ct_src)
            nc.sync.dma_start(out=st2[:, 0:gk0, :], in_=st_src)
            # generate the low-frequency table chunks on the ACT engine:
            # table[p, c] = Sin(scale_p * c + phi_p) with phi range-reduced
            # at the wave's center column on the host
            for kc in range(gk0, half):
                for tau in range(2):  # 0 = cos half, 1 = sin half
                    col = (w * 2 + tau) * ng + (kc - gk0)
                    nc.scalar.activation(
                        cs2[:, tau * half + kc, :],
                        iota2[:, 0:wsz],
                        mybir.ActivationFunctionType.Sin,
                        bias=gphi[:, col : col + 1],
                        scale=gsc[:, kc - gk0 : kc - gk0 + 1],
                    )
            if w == 0:
                make_upper_triangular(nc, umask, val=1.0, diag=False)
            # V arrives incrementally (wave-blocked staging): phase B of
            # wave w only reads row blocks jj < c1/P, so each wave loads
            # just the blocks the NEXT wave needs, behind qo on the
            # SWDGE queue.
            jv0, jv1 = vsched[w]
            if jv1 > jv0:
                nc.gpsimd.dma_start(
                    out=vb3[:, jv0:jv1, :],
                    in_=v_d[jv0 * P * d : jv1 * P * d].rearrange(
                        "(p j dd) -> p j dd", p=P, j=jv1 - jv0
                    ),
                )
            # RoPE, batched across all pair chunks (6 big DVE ops per wave):
            #   QRe = Qe*c - Qo*s -> qrt chunks [0, half)
            #   QRo = Qo*c + Qe*s -> qrt chunks [half, nk)
            t1 = tpool.tile([P, half, wsz], BF16, tag="t1", name=f"t1_{w}")
            t2 = tpool.tile([P, half, wsz], BF16, tag="t2", name=f"t2_{w}")
            nc.vector.tensor_mul(t1, qe2, ct2)
            nc.vector.tensor_mul(t2, qo2, st2)
            nc.vector.tensor_sub(qrt3[:, 0:half, c0:c1], t1, t2)
            t3 = tpool.tile([P, half, wsz], BF16, tag="t1", name=f"t3_{w}")
            t4 = tpool.tile([P, half, wsz], BF16, tag="t2", name=f"t4_{w}")
            nc.vector.tensor_mul(t3, qo2, ct2)
            nc.vector.tensor_mul(t4, qe2, st2)
            nc.vector.tensor_add(qrt3[:, half:nk, c0:c1], t3, t4)

            # ---- phase A: score strip tiles landing in wave w ------------
            for j in range(c1 // P):
                lo = max(j * P, c0)
                hi = c1
                width = hi - lo
                ps = spsum.tile([P, width], F32, tag="ps", name=f"ps_{w}_{j}")
                for k in range(nk):
                    nc.tensor.matmul(
                        ps,
                        lhsT=qrt[:, k * t + j * P : k * t + (j + 1) * P],
                        rhs=qrt[:, k * t + lo : k * t + hi],
                        start=(k == 0),
                        stop=(k == nk - 1),
                    )
                l0 = lo - j * P
                if l0 == 0:
                    # diagonal block: strict upper triangle in [s,t]
                    nc.vector.tensor_mul(strips[j][:, 0:P], ps[:, 0:P], umask)
                    if width > P:
                        nc.scalar.copy(strips[j][:, P:width], ps[:, P:width])
                else:
                    nc.scalar.copy(strips[j][:, l0 : l0 + width], ps[:, :width])

            # ---- phase B: outputs for row blocks of wave w ---------------
            for i in range(c0 // P, c1 // P):
                po = opsum.tile([P, d], F32, tag="po", name=f"po_{i}")
                for jj in range(i + 1):
                    nc.tensor.matmul(
                        po,
                        lhsT=strips[jj][:, (i - jj) * P : (i - jj + 1) * P],
                        rhs=vb[:, jj * d : (jj + 1) * d],
                        start=(jj == 0),
                        stop=(jj == i),
                    )
                ot = outp.tile([P, d], F32, tag="ot", name=f"ot_{i}")
                nc.scalar.copy(ot, po)
                nc.scalar.dma_start(out=out_d[i * P : (i + 1) * P, :], in_=ot)

    nc.compile()
    return nc


def _stage_q(Qc):
    """(t, n) f32 -> flat bf16 staging: de-interleaved (Qe^T rows over
    Qo^T rows), transposed, wave-blocked."""
    t, n = Qc.shape
    qp = np.ascontiguousarray(Qc.reshape(t, n // 2, 2).transpose(2, 1, 0))
    qt = qp.reshape(n, t).astype(ml_dtypes.bfloat16)
    return _wave_block(qt, _wave_bounds(t))


def _stage_v(Vc):
    """(t, d) f32 -> flat bf16 staging, wave-blocked per _v_sched."""
    t, d = Vc.shape
    bounds = _wave_bounds(t)
    out = np.empty(t * d, dtype=ml_dtypes.bfloat16)
    for j0, j1 in _v_sched(bounds, t // P):
        if j1 > j0:
            blk = (
                Vc[j0 * P : j1 * P, :]
                .astype(ml_dtypes.bfloat16)
                .reshape(j1 - j0, P, d)
                .transpose(1, 0, 2)
            )
            out[j0 * P * d : j1 * P * d] = blk.reshape(-1)
    return out


def _run(Q, V, trace=False, **trace_kwargs):
    Q = np.asarray(Q, dtype=np.float32)
    V = np.asarray(V, dtype=np.float32)
    b, h, t, n = Q.shape
    d = V.shape[-1]
    ncores = b * h
    nc = _build(t, n, d)
    cs = _rope_tables(t, n)
    gen = {}
    if _gen_k0(t, n) < n // (2 * P):
        iota, gsc, gphi = _gen_tables(t, n)
        gen = {"giota": iota, "gsc": gsc, "gphi": gphi}
    in_maps = []
    for core in range(ncores):
        bi, hi = divmod(core, h)
        in_maps.append(
            {
                "qt": _stage_q(Q[bi, hi]),
                "v": _stage_v(V[bi, 0]),
                "cs": cs,
                **gen,
            }
        )
    res = run_bass_kernel_spmd(
        nc, in_maps, core_ids=list(range(ncores)), trace=trace, **trace_kwargs
    )
    out = np.empty((b, h, t, d), dtype=np.float32)
    for core in range(ncores):
        bi, hi = divmod(core, h)
        out[bi, hi] = res.results[core]["out"]
    return out, res


def kernel(**inputs):
    out, _ = _run(inputs["Q"], inputs["V"], trace=False)
    return out
